# revision 28
# baseline (speedup 1.0000x reference)
"""Multi-head self-attention (B=4, N=2048, D=1024, H=16) on 8 trn2 NeuronCores.

Sharding: 8 shards = (batch, head-half).  Core c handles batch c//2 and heads
(c%2)*8 .. (c%2)*8+8, computing attention for ALL 2048 queries of its 8 heads.
After normalization, an AllToAll within each core pair exchanges attention
outputs so each rank ends with all 16 heads for its own query half (rank r of
the pair keeps queries [r*1024, (r+1)*1024)), then runs the output projection
on that half.  No K/V duplication, all matmuls bf16.

Per-core kernel (Tile):
  1. PE-transpose z -> ztc (din-major) bf16.
  2. V (natural, per key-chunk/head), K^T/Q^T (dout-major) via bf16 matmuls.
  3. Per head-pair: scores S^T = K Q^T with two 64-contraction row-tiled
     matmuls running concurrently (tile_position (0,0)/(64,0)); exp on ACT;
     P^T V via two col-tiled matmuls (outputs partitions 0-63/64-127);
     softmax denominators via ones-vector matmuls col-tiled at (0,0)/(0,32);
     reciprocal + gpsimd partition-broadcast; normalization fused into the
     PSUM->SBUF evacuation.
  4. AllToAll per pair, then final projection + bias on own query half.
"""

import os
import sys

_TRN_REPO = "/opt/trn_rl_repo"
if os.path.isdir(_TRN_REPO) and _TRN_REPO not in sys.path:
    sys.path.insert(0, _TRN_REPO)

import ml_dtypes
import numpy as np

import concourse.bass as bass  # noqa: E402
import concourse.mybir as mybir  # noqa: E402
from concourse import bacc  # noqa: E402
from concourse.bass_utils import run_bass_kernel_spmd  # noqa: E402
from concourse.masks import make_identity  # noqa: E402
from concourse.tile import TileContext  # noqa: E402

F32 = mybir.dt.float32
BF16 = mybir.dt.bfloat16
MULT = mybir.AluOpType.mult
ADD = mybir.AluOpType.add
EXP = mybir.ActivationFunctionType.Exp
BF = ml_dtypes.bfloat16

N_CORES = 8
B, N, D = 4, 2048, 1024
H, HD = 16, 64
NQ = N // 2          # query rows per core output
P = 128
DC = D // P          # 8 din chunks of 128
DH = D // 2          # 512 dout per core for q/k/v
NP = 4               # head pairs per core
NKC = N // P         # 16 key chunks
NQH = 4              # query chunks of 512 in phase 2
SCALE = 1.0 / 8.0    # 1/sqrt(HD)
GROUPS = [[0, 1], [2, 3], [4, 5], [6, 7]]


def _build():
    nc = bacc.Bacc("TRN2", target_bir_lowering=False, debug=False,
                   num_devices=N_CORES)
    z_d = nc.declare_dram_parameter("z", [N, D], BF16, isOutput=False)
    wq_d = nc.declare_dram_parameter("w_q", [D, DH], BF16, isOutput=False)
    wk_d = nc.declare_dram_parameter("w_k", [D, DH], BF16, isOutput=False)
    wv_d = nc.declare_dram_parameter("w_v", [D, DH], BF16, isOutput=False)
    wo_d = nc.declare_dram_parameter("w_o", [D, D], BF16, isOutput=False)
    bo_d = nc.declare_dram_parameter("b_o", [D], F32, isOutput=False)
    # qmask: 1.0 if this core keeps query half 0, else 0.0 (uniform program,
    # per-core data selects the half after the pairwise AllGather).
    qm_d = nc.declare_dram_parameter("qmask", [P, NQ], mybir.dt.uint8,
                                    isOutput=False)
    out_d = nc.declare_dram_parameter("out", [NQ, D], F32, isOutput=True)
    dbg_d = None
    if os.environ.get("MHA_DEBUG_TAP"):
        dbg_d = nc.declare_dram_parameter("dbg", [P, 36, N], BF16,
                                          isOutput=True)

    with TileContext(nc) as tc:
        with tc.tile_pool(name="const", bufs=1) as constp, \
             tc.tile_pool(name="pers", bufs=1) as pers, \
             tc.tile_pool(name="dram", bufs=1, space="DRAM") as dram:
            ident = constp.tile([P, P], BF16)
            make_identity(nc, ident)
            ones = constp.tile([P, 1], BF16)
            nc.vector.memset(ones[:], 1.0)

            # persistent SBUF tensors
            ztc = pers.tile([P, DC, N], BF16)          # z^T
            kt = pers.tile([P, NP, N], BF16)           # K^T (pair-major)
            qt = pers.tile([P, NP, N], BF16)           # Q^T
            vp = pers.tile([P, NKC, 8, HD], BF16)      # V natural per kc/head
            attnU = pers.tile([P, NP, N], BF16)        # normalized attn^T
            attnG = pers.tile([P, DC, NQ], BF16)       # gathered, own q half
            # denominators: partition 64*(p%2)+32*par, free (p//2, qh).
            # Engine partition bases must be 32-aligned; reciprocal runs
            # in place over a 33-partition slice (junk rows preset to 1.0).
            den_sb = pers.tile([P, 2, NQH, 512], F32)
            nc.vector.memset(den_sb[:], 1.0)

            wq_sb = pers.tile([P, DC, DH], BF16)
            wk_sb = pers.tile([P, DC, DH], BF16)
            wv_sb = pers.tile([P, DC, DH], BF16)
            wo_sb = pers.tile([P, DC, D], BF16)
            bo_sb = pers.tile([1, D], F32)
            bo_bc = pers.tile([P, D], F32)
            qm_sb = pers.tile([P, NQ], mybir.dt.uint8)
            nc.sync.dma_start(qm_sb[:], qm_d[:])

            nc.sync.dma_start(wk_sb[:], wk_d.rearrange("(c p) o -> p c o", p=P))
            nc.sync.dma_start(wq_sb[:], wq_d.rearrange("(c p) o -> p c o", p=P))
            nc.sync.dma_start(wv_sb[:], wv_d.rearrange("(c p) o -> p c o", p=P))
            nc.sync.dma_start(wo_sb[:], wo_d.rearrange("(c p) o -> p c o", p=P))
            nc.sync.dma_start(bo_sb[:], bo_d[None, :])
            nc.gpsimd.partition_broadcast(bo_bc[:], bo_sb[:])

            # collective bounce buffers (per pair)
            in_bs = [dram.tile([P, N], BF16, name=f"inb{i}")
                     for i in range(NP)]
            out_bs = [dram.tile([2, P, N], BF16, name=f"outb{i}")
                      for i in range(NP)]

            # ---------- Phase 0: transpose z ----------
            with tc.tile_pool(name="zin",
                              bufs=1 if dbg_d is not None else 2) as zinp, \
                 tc.tile_pool(name="pst", bufs=2, space="PSUM") as pst:
                for t in range(4):
                    zt_in = zinp.tile([P, 4, D], BF16, name="zt_in")
                    nc.sync.dma_start(
                        zt_in[:],
                        z_d[t * 512:(t + 1) * 512, :].rearrange(
                            "(r p) d -> p r d", p=P))
                    for dc in range(DC):
                        # full-bank tile (bf16 512 is half a bank; two
                        # half-bank bufs sharing a bank -> PE/DVE collision)
                        ps = pst.tile([P, 1024], BF16)
                        for r in range(4):
                            nc.tensor.transpose(
                                ps[:, r * P:(r + 1) * P],
                                zt_in[:, r, dc * P:(dc + 1) * P],
                                ident[:])
                        nc.vector.tensor_copy(
                            ztc[:, dc, t * 512:(t + 1) * 512], ps[:, 0:512])

            # K^T / Q^T group helper (pool passed per phase)
            def kq_group(pool, p, which, col):
                ps = pool.tile([P, 512], F32, name="kq")
                w = wk_sb if which == 0 else wq_sb
                dst = kt if which == 0 else qt
                for dc in range(DC):
                    nc.tensor.matmul(
                        ps[:],
                        lhsT=w[:, dc, p * P:(p + 1) * P],
                        rhs=ztc[:, dc, col * 512:(col + 1) * 512],
                        start=(dc == 0), stop=(dc == DC - 1))
                st = dst[:, p, col * 512:(col + 1) * 512]
                nc.vector.tensor_copy(st, ps[:])

            # ---------- Phase 1a: V projection (all 8 heads) ----------
            with tc.tile_pool(name="vproj", bufs=3, space="PSUM") as vproj:
                for kc in range(NKC):
                    ps = vproj.tile([P, DH], F32)
                    for dc in range(DC):
                        nc.tensor.matmul(
                            ps[:],
                            lhsT=ztc[:, dc, kc * P:(kc + 1) * P],
                            rhs=wv_sb[:, dc, :],
                            start=(dc == 0), stop=(dc == DC - 1))
                    nc.vector.tensor_copy(
                        vp[:, kc, :, :],
                        ps.rearrange("p (h d) -> p h d", d=HD))

                # K^T / Q^T for all pairs
                for p in range(NP):
                    for col in range(4):
                        kq_group(vproj, p, 0, col)
                    for col in range(4):
                        kq_group(vproj, p, 1, col)

            if dbg_d is not None:
                nc.sync.dma_start(dbg_d[:, 0:8, :], ztc[:])
                nc.sync.dma_start(dbg_d[:, 8:12, :], kt[:])
                nc.sync.dma_start(dbg_d[:, 12:16, :], qt[:])
                nc.sync.dma_start(
                    dbg_d[:, 16:20, :],
                    vp.rearrange("p a b c -> p (a b c)").rearrange(
                        "p (s n) -> p s n", s=4))

            # ---------- Phase 2: attention per head pair ----------
            if True:
                with tc.tile_pool(name="ss", bufs=2, space="PSUM") as ssp, \
                     tc.tile_pool(name="pv", bufs=1, space="PSUM") as pvp, \
                     tc.tile_pool(name="dn", bufs=1, space="PSUM") as dnp, \
                     tc.tile_pool(name="es", bufs=3) as esp, \
                     tc.tile_pool(name="ag", bufs=1) as agp, \
                     tc.tile_pool(name="rb", bufs=1) as rbp:
                    for p in range(NP):
                        h0, h1 = 2 * p, 2 * p + 1
                        for qh in range(NQH):
                            # separate banks per accumulation chain: a
                            # start=True clears has_written for its WHOLE
                            # bank, so interleaved chains must not share one.
                            pv = pvp.tile([P, 2, 512], F32, name="pv")
                            den = dnp.tile([P, 2, 512], F32, name="dn")
                            for kc in range(NKC):
                                ss = ssp.tile([P, 2, 512], F32, name="ss")
                                es = esp.tile([P, 2, 512], BF16, name="es")
                                nc.tensor.matmul(
                                    ss[:, 0, :],
                                    lhsT=kt[0:64, p, kc * P:(kc + 1) * P],
                                    rhs=qt[0:64, p, qh * 512:(qh + 1) * 512])
                                nc.tensor.matmul(
                                    ss[:, 1, :],
                                    lhsT=kt[64:P, p, kc * P:(kc + 1) * P],
                                    rhs=qt[64:P, p, qh * 512:(qh + 1) * 512])
                                nc.scalar.activation(es[:], ss[:], EXP,
                                                     scale=SCALE)
                                if dbg_d is not None and p == 0 and qh == 0 \
                                        and kc == 0:
                                    es0 = pers.tile([P, 2, 512], BF16)
                                    ss0 = pers.tile([P, 2, 512], BF16)
                                    nc.vector.tensor_copy(es0[:], es[:])
                                    nc.vector.tensor_copy(ss0[:], ss[:])
                                    nc.sync.dma_start(
                                        dbg_d[:, 30, 0:1024],
                                        es0.rearrange("p a b -> p (a b)"))
                                    nc.sync.dma_start(
                                        dbg_d[:, 31, 0:1024],
                                        ss0.rearrange("p a b -> p (a b)"))
                                nc.tensor.matmul(
                                    pv[0:HD, 0, :], lhsT=vp[:, kc, h0, :],
                                    rhs=es[:, 0, :],
                                    start=(kc == 0), stop=(kc == NKC - 1))
                                nc.tensor.matmul(
                                    pv[HD:P, 1, :], lhsT=vp[:, kc, h1, :],
                                    rhs=es[:, 1, :],
                                    start=(kc == 0), stop=(kc == NKC - 1))
                                nc.tensor.matmul(
                                    den[0:1, 0, :], lhsT=ones[:],
                                    rhs=es[:, 0, :],
                                    start=(kc == 0), stop=(kc == NKC - 1))
                                nc.tensor.matmul(
                                    den[32:33, 1, :], lhsT=ones[:],
                                    rhs=es[:, 1, :],
                                    start=(kc == 0), stop=(kc == NKC - 1))
                            # denominators -> reciprocal -> normalize + evac
                            b0, pg = 64 * (p % 2), p // 2
                            nc.vector.tensor_copy(
                                den_sb[b0:b0 + 1, pg, qh, :], den[0:1, 0, :])
                            nc.vector.tensor_copy(
                                den_sb[b0 + 32:b0 + 33, pg, qh, :],
                                den[32:33, 1, :])
                            nc.vector.reciprocal(
                                den_sb[b0:b0 + 33, pg, qh, :],
                                den_sb[b0:b0 + 33, pg, qh, :])
                            r0 = rbp.tile([1, 512], F32, name="r0")
                            r1 = rbp.tile([1, 512], F32, name="r1")
                            nc.vector.tensor_copy(
                                r0[:], den_sb[b0:b0 + 1, pg, qh, :])
                            nc.vector.tensor_copy(
                                r1[:], den_sb[b0 + 32:b0 + 33, pg, qh, :])
                            rbA = rbp.tile([P, 512], F32, name="rbA")
                            rbB = rbp.tile([P, 512], F32, name="rbB")
                            nc.gpsimd.partition_broadcast(rbA[:], r0[:])
                            nc.gpsimd.partition_broadcast(rbB[:], r1[:])
                            if dbg_d is not None and p == 0 and qh == 0:
                                pv16 = pers.tile([P, 2, 512], BF16)
                                rb16 = pers.tile([P, 512], BF16)
                                nc.vector.tensor_copy(pv16[:], pv[:])
                                nc.vector.tensor_copy(
                                    rb16[0:HD, :], rbA[0:HD, :])
                                nc.vector.tensor_copy(
                                    rb16[HD:P, :], rbB[HD:P, :])
                                nc.sync.dma_start(
                                    dbg_d[:, 32, 0:1024],
                                    pv16.rearrange("p a b -> p (a b)"))
                                nc.sync.dma_start(dbg_d[:, 33, 0:512], rb16[:])
                            nc.vector.tensor_tensor(
                                attnU[0:HD, p, qh * 512:(qh + 1) * 512],
                                pv[0:HD, 0, :], rbA[0:HD, :], MULT)
                            nc.vector.tensor_tensor(
                                attnU[HD:P, p, qh * 512:(qh + 1) * 512],
                                pv[HD:P, 1, :], rbB[HD:P, :], MULT)
                        # exchange pair p with partner core via AllGather,
                        # then keep own query half (qmask selects).
                        nc.sync.dma_start(in_bs[p][:], attnU[:, p, :])
                        nc.gpsimd.collective_compute(
                            "AllGather",
                            mybir.AluOpType.bypass,
                            replica_groups=GROUPS,
                            ins=[in_bs[p].opt()],
                            outs=[out_bs[p].opt()],
                        )
                        ag = agp.tile([P, 2, N], BF16, name="ag")
                        nc.sync.dma_start(
                            ag[:], out_bs[p].rearrange("s p q -> p s q"))
                        for r in range(2):
                            nc.vector.select(
                                attnG[:, r * NP + p, :],
                                qm_sb[:],
                                ag[:, r, 0:NQ],
                                ag[:, r, NQ:N])

            if dbg_d is not None:
                nc.sync.dma_start(dbg_d[:, 20:24, :], attnU[:])
                nc.sync.dma_start(
                    dbg_d[:, 24:28, :],
                    attnG.rearrange("p a b -> p (a b)").rearrange(
                        "p (s n) -> p s n", s=4))

            # ---------- Phase 3: output projection + bias ----------
            with tc.tile_pool(name="ot", bufs=4) as outp, \
                 tc.tile_pool(name="psf", bufs=3, space="PSUM") as fpp:
                for q8 in range(NQ // P):
                    psf = fpp.tile([P, D], F32, name="pf")
                    for dc in range(DC):
                        lh = attnG[:, dc, q8 * P:(q8 + 1) * P]
                        for oc2 in range(2):
                            nc.tensor.matmul(
                                psf[:, oc2 * 512:(oc2 + 1) * 512],
                                lhsT=lh,
                                rhs=wo_sb[:, dc, oc2 * 512:(oc2 + 1) * 512],
                                start=(dc == 0), stop=(dc == DC - 1))
                    ot = outp.tile([P, D], F32)
                    nc.vector.tensor_tensor(ot[:], psf[:], bo_bc[:], ADD)
                    nc.sync.dma_start(out_d[q8 * P:(q8 + 1) * P, :], ot[:])

    nc.compile()
    return nc


_NC_CACHE = None


def _get_nc():
    global _NC_CACHE
    if _NC_CACHE is None:
        _NC_CACHE = _build()
    return _NC_CACHE


def _run(z, w_q, w_k, w_v, w_o, b_o, **spmd_kwargs):
    z = np.asarray(z, dtype=np.float32)
    w_q = np.asarray(w_q, dtype=np.float32)
    w_k = np.asarray(w_k, dtype=np.float32)
    w_v = np.asarray(w_v, dtype=np.float32)
    w_o = np.asarray(w_o, dtype=np.float32)
    b_o = np.ascontiguousarray(np.asarray(b_o, dtype=np.float32))
    assert z.shape == (B, N, D)

    if not spmd_kwargs.get("trace"):
        os.environ["BASS_NEVER_TRACE"] = "1"

    nc = _get_nc()
    z16 = [np.ascontiguousarray(z[b].astype(BF)) for b in range(B)]
    wo16 = np.ascontiguousarray(w_o.astype(BF))
    wq16 = w_q.astype(BF)
    wk16 = w_k.astype(BF)
    wv16 = w_v.astype(BF)
    qmasks = [np.full((P, NQ), 1 - hc, dtype=np.uint8) for hc in range(2)]
    in_maps = []
    for c in range(N_CORES):
        b, hc = c // 2, c % 2
        sl = slice(hc * DH, (hc + 1) * DH)
        in_maps.append({
            "z": z16[b],
            "w_q": np.ascontiguousarray(wq16[:, sl]),
            "w_k": np.ascontiguousarray(wk16[:, sl]),
            "w_v": np.ascontiguousarray(wv16[:, sl]),
            "w_o": wo16,
            "b_o": b_o,
            "qmask": qmasks[hc],
        })

    res = run_bass_kernel_spmd(nc, in_maps, core_ids=list(range(N_CORES)),
                               **spmd_kwargs)
    out = np.empty((B, N, D), dtype=np.float32)
    for c in range(N_CORES):
        b, hc = c // 2, c % 2
        out[b, hc * NQ:(hc + 1) * NQ, :] = res.results[c]["out"]
    return out, res


def kernel(z, w_q, w_k, w_v, w_o, b_o):
    out, _ = _run(z, w_q, w_k, w_v, w_o, b_o)
    return out
